# revision 19
# baseline (speedup 1.0000x reference)
"""MultiHeadAttention (B=2, S=4096, D=512, H=8) on 8 TRN2 NeuronCores.

Sharding: core c -> batch b = c//4, head-pair hp = c%4 (heads 2*hp, 2*hp+1).
Each core computes the partial output  concat(O_h0,O_h1) @ Wo[:,128cols].T
for its batch; host sums the 4 partials per batch and adds bo + Wo@bv.

Math notes (exact rewrites, not approximations):
  - K-bias bk drops out of softmax (adds a per-query constant to scores).
  - V-bias bv passes through softmax unchanged -> becomes the constant
    Wo@bv added on the host.
  - 1/sqrt(64) is folded into Wq and bq on the host.
Device dataflow per (head, 1024-query group):
  scoresT[k,q] = (K-chunk @ QT) in PSUM  (keys on partitions)
  probsT = exp(scoresT) via ScalarE (no max subtraction: |scores| < ~2.5)
  [OT; sums] accumulated in PSUM via lhsT=[V_chunk | ones] (M=65)
  normalization happens after the output projection, where q is the
  partition axis so 1/sums is a legal per-partition scalar.
"""

import os
import sys

sys.path.insert(0, "/opt/trn_rl_repo")

import numpy as np
import ml_dtypes

import concourse.bass as bass
import concourse.bacc as bacc
import concourse.tile as tile
import concourse.mybir as mybir
import concourse.bass_utils as bass_utils

BF16 = ml_dtypes.bfloat16
F32 = np.float32
DT = mybir.dt

S = 4096
D = 512
HD = 64
NCORES = 8

_CACHE = {}


def _build_module():
    nc = bacc.Bacc(
        "TRN2",
        target_bir_lowering=False,
        debug=False,
        enable_asserts=False,
        num_devices=NCORES,
    )
    xT_d = nc.dram_tensor("xT", (D, S), DT.bfloat16, kind="ExternalInput").ap()
    wqT_d = nc.dram_tensor("wqT", (D, 128), DT.bfloat16, kind="ExternalInput").ap()
    wkT_d = nc.dram_tensor("wkT", (D, 128), DT.bfloat16, kind="ExternalInput").ap()
    wvT_d = nc.dram_tensor("wvT", (D, 128), DT.bfloat16, kind="ExternalInput").ap()
    bq_d = nc.dram_tensor("bq", (128, 1), DT.float32, kind="ExternalInput").ap()
    woT_d = nc.dram_tensor("woT", (128, D), DT.bfloat16, kind="ExternalInput").ap()
    out_d = nc.dram_tensor("out", (S, D), DT.float32, kind="ExternalOutput").ap()

    with tile.TileContext(nc) as tc:
        with (
            tc.tile_pool(name="const", bufs=1) as cpool,
            tc.tile_pool(name="probs", bufs=6) as ppool,
            tc.tile_pool(name="tmp", bufs=4) as tpool,
            tc.tile_pool(name="osb", bufs=4) as opool,
            tc.tile_pool(name="psum", bufs=2, space="PSUM") as psum,
        ):
            # ---- load constants (weights first; x^T in 8 column slices
            # so the first projection can start after ~0.5MB lands) ----
            wq = cpool.tile([128, 4, 128], DT.bfloat16)
            wk = cpool.tile([128, 4, 128], DT.bfloat16)
            wv = cpool.tile([128, 4, 128], DT.bfloat16)
            xt = cpool.tile([128, 4, S], DT.bfloat16)  # x^T, contraction-chunked
            bqs = cpool.tile([128, 1], DT.float32)
            wo = cpool.tile([128, D], DT.bfloat16)

            def xt_slice(sb):
                nc.sync.dma_start(
                    xt[:, :, sb * 512 : (sb + 1) * 512],
                    xT_d[:, sb * 512 : (sb + 1) * 512].rearrange(
                        "(c p) m -> p c m", p=128
                    ),
                )

            nc.sync.dma_start(wk[:], wkT_d.rearrange("(c p) m -> p c m", p=128))
            xt_slice(0)
            nc.sync.dma_start(wq[:], wqT_d.rearrange("(c p) m -> p c m", p=128))
            nc.sync.dma_start(bqs[:], bq_d[:])
            xt_slice(1)
            nc.sync.dma_start(wv[:], wvT_d.rearrange("(c p) m -> p c m", p=128))
            nc.sync.dma_start(wo[:], woT_d[:])
            for sb in range(2, 8):
                xt_slice(sb)

            # ---- persistent SBUF tensors ----
            qt = cpool.tile([128, S], DT.bfloat16)  # Q^T (2 heads stacked)
            kt = cpool.tile([128, S], DT.bfloat16)  # K^T
            # V in natural [k, d] layout + a ones column per head:
            # [128, kchunk, 2*65]; col 64/129 are the ones columns.
            vext = cpool.tile([128, 32, 130], DT.bfloat16)
            otn = cpool.tile([128, S], DT.bfloat16)  # unnormalized O^T
            recip2 = cpool.tile([128, 64], DT.float32)  # transposed 1/sums
            # softmax denominators; head h lives on partition 32*h (engine
            # APs must start at a 32-aligned partition)
            sums = cpool.tile([33, S], DT.float32)
            sumsT = cpool.tile([128, 64], DT.float32)  # sums, q on partitions

            # ---- projections (interleaved into the attention stream) ----
            def proj_block(dst, w, bias, sb):
                pt = psum.tile([128, 512], DT.float32, tag="sc", name="pt")
                for kc in range(4):
                    nc.tensor.matmul(
                        pt[:],
                        w[:, kc, :],
                        xt[:, kc, sb * 512 : (sb + 1) * 512],
                        start=(kc == 0),
                        stop=(kc == 3),
                    )
                if bias is not None:
                    nc.vector.tensor_scalar(
                        dst[:, sb * 512 : (sb + 1) * 512],
                        pt[:],
                        bias[:, 0:1],
                        None,
                        mybir.AluOpType.add,
                    )
                else:
                    nc.vector.tensor_copy(dst[:, sb * 512 : (sb + 1) * 512], pt[:])

            vsp = vext[:].rearrange("p k (a b) -> p k a b", a=2)

            def v_block(kb):
                pt = psum.tile([128, 128], DT.float32, tag="acc", name="pt_v")
                for kc in range(4):
                    nc.tensor.matmul(
                        pt[:],
                        xt[:, kc, kb * 128 : (kb + 1) * 128],
                        wv[:, kc, :],
                        start=(kc == 0),
                        stop=(kc == 3),
                    )
                nc.vector.tensor_copy(
                    vsp[:, kb, :, 0:64],
                    pt[:].rearrange("p (a b) -> p a b", a=2),
                )
                nc.vector.memset(vsp[:, kb, :, 64:65], 1.0)

            # minimal prefix so ScalarE starts exp'ing almost immediately
            proj_block(kt, wk, None, 0)
            proj_block(qt, wq, bqs, 0)
            proj_block(qt, wq, bqs, 1)

            # ---- attention, query-group-major, software-pipelined ----
            # scores+exp of chunk i are emitted before attn@V of chunk i-1 so
            # PE never stalls on ACT round-trips. K/V projection blocks are
            # spread through (g0,h0); output projection for group g is spread
            # through (g+1,h0), 1 block per 4 chunks.
            chunks = [(g, h, kc) for g in range(4) for h in range(2) for kc in range(32)]
            acc_t = {}
            pb_t = {}
            t0_t = {}
            pending_a = {}
            pending_finals = {}

            def emit_scores_exp(g, h, kc):
                if g == 0 and h == 0 and 1 <= kc <= 7:
                    proj_block(kt, wk, None, kc)  # K block sb first used at chunk 4*sb
                if h == 1 and g <= 2 and kc in (20, 26):
                    proj_block(qt, wq, bqs, 2 * (g + 1) + (kc == 26))
                hr = h * 64
                q0 = g * 1024
                sc = psum.tile([128, 1024], DT.float32, tag="sc", name="sc")
                for qh in range(2):
                    nc.tensor.matmul(
                        sc[:, qh * 512 : (qh + 1) * 512],
                        kt[hr : hr + 64, kc * 128 : (kc + 1) * 128],
                        qt[hr : hr + 64, q0 + qh * 512 : q0 + (qh + 1) * 512],
                        start=True,
                        stop=True,
                    )
                pb = ppool.tile([128, 1024], DT.bfloat16, name="pb")
                nc.scalar.activation(pb[:], sc[:], mybir.ActivationFunctionType.Exp)
                pb_t[(g, h, kc)] = pb

            def emit_av(g, h, kc):
                if g == 0 and h == 0:
                    v_block(kc)  # V chunk kb first used right here
                pb = pb_t.pop((g, h, kc))
                if kc == 0:
                    acc_t[(g, h)] = psum.tile(
                        [65, 1024], DT.float32, tag="acc", name="acc"
                    )
                acc = acc_t[(g, h)]
                for qh in range(2):
                    nc.tensor.matmul(
                        acc[:, qh * 512 : (qh + 1) * 512],
                        vext[:, kc, h * 65 : h * 65 + 65],
                        pb[:, qh * 512 : (qh + 1) * 512],
                        start=(kc == 0),
                        stop=(kc == 31),
                    )
                if kc == 31:
                    emit_drain(g, h)
                if h == 0 and kc % 4 == 3 and pending_finals.get(g - 1):
                    emit_final_block(*pending_finals[g - 1].pop(0))

            def emit_drain(g, h):
                hr = h * 64
                q0 = g * 1024
                acc = acc_t.pop((g, h))
                nc.vector.tensor_copy(otn[hr : hr + 64, q0 : q0 + 1024], acc[0:64, :])
                nc.vector.tensor_copy(
                    sums[h * 32 : h * 32 + 1, q0 : q0 + 1024], acc[64:65, :]
                )
                # transpose the sums row, one 128-q column per DMA, spread
                # across 4 engine DMA queues so they overlap
                c0 = h * 32 + g * 8
                tail = g == 3 and h == 1  # ACT idle at the tail: use its queue too
                for i in range(8):
                    s0 = q0 + i * 128
                    eng = (nc.sync, nc.scalar)[i % 2] if tail else nc.sync
                    eng.dma_start(
                        sumsT[:, c0 + i : c0 + i + 1],
                        sums[h * 32 : h * 32 + 1, s0 : s0 + 128].rearrange(
                            "p (j o) -> p j o", o=1
                        ),
                    )
                    if i == 3:  # let the first final blocks start early
                        nc.vector.reciprocal(
                            recip2[:, c0 : c0 + 4], sumsT[:, c0 : c0 + 4]
                        )
                nc.vector.reciprocal(recip2[:, c0 + 4 : c0 + 8], sumsT[:, c0 + 4 : c0 + 8])
                if h == 1:
                    pending_finals[g] = [(g, i) for i in range(8)]

            def emit_final_block(g, i):
                r0 = (g * 8 + i) * 128
                p0 = psum.tile([128, 512], DT.float32, tag="sc", name="p0")
                p1 = psum.tile([128, 512], DT.float32, tag="acc", name="p1")
                nc.tensor.matmul(
                    p0[:], otn[0:64, r0 : r0 + 128], wo[0:64, :],
                    start=True, stop=True,
                )
                nc.tensor.matmul(
                    p1[:], otn[64:128, r0 : r0 + 128], wo[64:128, :],
                    start=True, stop=True,
                )
                t0 = tpool.tile([128, 512], DT.float32, tag="t0", name="t0")
                nc.vector.tensor_scalar(
                    t0[:], p0[:], recip2[:, g * 8 + i : g * 8 + i + 1], None,
                    mybir.AluOpType.mult,
                )
                o = opool.tile([128, 512], DT.float32, name="o")
                nc.vector.scalar_tensor_tensor(
                    o[:], p1[:], recip2[:, 32 + g * 8 + i : 32 + g * 8 + i + 1],
                    t0[:], mybir.AluOpType.mult, mybir.AluOpType.add,
                )
                eng = (nc.sync, nc.scalar)[i % 2] if g == 3 else nc.sync
                eng.dma_start(out_d[r0 : r0 + 128, :], o[:])

            for i in range(len(chunks) + 1):
                if i < len(chunks):
                    emit_scores_exp(*chunks[i])
                if i > 0:
                    emit_av(*chunks[i - 1])
            for g, i in pending_finals.get(3, []):
                emit_final_block(g, i)

    nc.compile()
    return nc


def _get_module():
    if "nc" not in _CACHE:
        _CACHE["nc"] = _build_module()
    return _CACHE["nc"]


def _prep_in_maps(x, Wq, bq, Wk, bk, Wv, bv, Wo, bo):
    in_maps = []
    wqT = np.ascontiguousarray((Wq / 8.0).T.astype(BF16))
    wkT = np.ascontiguousarray(Wk.T.astype(BF16))
    wvT = np.ascontiguousarray(Wv.T.astype(BF16))
    woT = np.ascontiguousarray(Wo.T.astype(BF16))
    bq8 = (bq / 8.0).astype(F32)
    xTb = [np.ascontiguousarray(x[b].T.astype(BF16)) for b in range(2)]
    for c in range(NCORES):
        b = c // 4
        js = slice((c % 4) * 128, (c % 4 + 1) * 128)
        in_maps.append(
            {
                "xT": xTb[b],
                "wqT": wqT[:, js],
                "wkT": wkT[:, js],
                "wvT": wvT[:, js],
                "bq": np.ascontiguousarray(bq8[js].reshape(128, 1)),
                "woT": np.ascontiguousarray(woT[js, :]),
            }
        )
    return in_maps


def kernel(x, Wq, bq, Wk, bk, Wv, bv, Wo, bo, _trace=False):
    x = np.asarray(x, dtype=np.float32)
    nc = _get_module()
    in_maps = _prep_in_maps(
        x, np.asarray(Wq), np.asarray(bq), np.asarray(Wk), np.asarray(bk),
        np.asarray(Wv), np.asarray(bv), np.asarray(Wo), np.asarray(bo),
    )
    res = None
    for attempt in range(3):
        try:
            res = bass_utils.run_bass_kernel_spmd(
                nc, in_maps, core_ids=list(range(NCORES)), trace=_trace
            )
            break
        except Exception:
            # transient NRT device wedge: retry with a freshly rebuilt module
            if attempt == 2:
                raise
            _CACHE.clear()
            nc = _get_module()
    const = (np.asarray(bo) + np.asarray(Wo) @ np.asarray(bv)).astype(F32)
    out = np.empty((2, S, D), dtype=np.float32)
    for b in range(2):
        acc = res.results[4 * b]["out"].astype(np.float64)
        for c in range(4 * b + 1, 4 * b + 4):
            acc = acc + res.results[c]["out"]
        out[b] = (acc + const).astype(np.float32)
    if _trace:
        return out, res
    return out
